# revision 50
# baseline (speedup 1.0000x reference)
"""GIN message-passing Actor network on 8 TRN2 NeuronCores (Bass/Tile).

Sharding: data-parallel over B=32 graphs -> 4 graphs (4096 nodes) per core.
Edges routed to the core owning dst, bucketed by 128-node dst tile.

v2 design:
- weighted one-hot (1/deg folded in) built ONCE via single tensor_scalar
  (is_equal+mult) ops, cached in SBUF bf16, reused across all 4 layers
- aggregation psum accumulates: edge-chunk matmuls (gathered bf16 rows
  stationary, cached one-hot moving) + h'^T via (h_nm stationary, IDENT
  moving) matmul (fuses the transpose) + rank-1 BN-offset correction
- BN deferred: AllGather h' (pre-BN) immediately after the MLP; BN stats
  AllReduce runs off critical path; the affine folds into the next layer's
  w1 (row scaling) and a rank-1 c-term; node_pool accumulates affine'd h'
- h' kept node-major (partition=node-in-tile): mm2 computed transposed
  (rt stationary, w2 moving), bias via rank-1; stats via Gram matmul
- all GIN matmuls bf16; policy MLP fp32r (full speed at 512-wide)
- output pi: broadcast-DMA zero-fill at t=0 + dma_scatter_add of the
  ~64 softmax nonzeros per graph (512B rows)
"""
import sys
sys.path.insert(0, '/opt/trn_rl_repo')
import numpy as np
import ml_dtypes
from concourse import bass, mybir, tile, bacc

import jax
import time
from jax.sharding import Mesh, PartitionSpec
from jax.experimental.shard_map import shard_map
from concourse import bass2jax
from concourse.bass2jax import _bass_exec_p, partition_id_tensor, install_neuronx_cc_hook


def build_exec(nc, n_cores, chain=1):
    install_neuronx_cc_hook()
    partition_name = nc.partition_id_tensor.name if nc.partition_id_tensor else None
    in_names, out_names, out_avals, zero_outs = [], [], [], []
    for alloc in nc.m.functions[0].allocations:
        if not isinstance(alloc, mybir.MemoryLocationSet):
            continue
        name = alloc.memorylocations[0].name
        if alloc.kind == "ExternalInput":
            if name != partition_name:
                in_names.append(name)
        elif alloc.kind == "ExternalOutput":
            shape = tuple(alloc.tensor_shape)
            dtype = mybir.dt.np(alloc.dtype)
            out_names.append(name)
            out_avals.append(jax.core.ShapedArray(shape, dtype))
            zero_outs.append(np.zeros(shape, dtype))
    n_params = len(in_names)
    all_in = list(in_names) + list(out_names)
    if partition_name is not None:
        all_in.append(partition_name)

    def _body(*args):
        operands = list(args)
        if partition_name is not None:
            operands.append(partition_id_tensor())
        for _ in range(chain):
            outs = _bass_exec_p.bind(
                *operands,
                out_avals=tuple(out_avals),
                in_names=tuple(all_in),
                out_names=tuple(out_names),
                lowering_input_output_aliases=(),
                sim_require_finite=True, sim_require_nnan=True, nc=nc,
            )
        return tuple(outs)

    devices = jax.devices()[:n_cores]
    mesh = Mesh(np.asarray(devices), ("core",))
    in_specs = (PartitionSpec("core"),) * (n_params + len(out_names))
    out_specs = (PartitionSpec("core"),) * len(out_names)
    sharded = jax.jit(shard_map(_body, mesh=mesh, in_specs=in_specs,
                                out_specs=out_specs, check_rep=False),
                      keep_unused=True)

    def run(in_maps, repeats=1):
        per_core = [[np.asarray(m[n]) for n in in_names] for m in in_maps]
        concat_in = [np.concatenate([per_core[c][i] for c in range(n_cores)], axis=0)
                     for i in range(n_params)]
        concat_zeros = [np.zeros((n_cores * z.shape[0], *z.shape[1:]), z.dtype) for z in zero_outs]
        times = []
        out_arrs = None
        for r in range(repeats):
            t0 = time.perf_counter()
            out_arrs = sharded(*concat_in, *concat_zeros)
            jax.block_until_ready(out_arrs)
            times.append(time.perf_counter() - t0)
        results = [
            {name: np.asarray(out_arrs[i]).reshape(n_cores, *out_avals[i].shape)[c]
             for i, name in enumerate(out_names)}
            for c in range(n_cores)
        ]
        return results, times
    return run


F32 = mybir.dt.float32
F32R = mybir.dt.float32r
BF16 = mybir.dt.bfloat16
I16 = mybir.dt.int16
AF = mybir.ActivationFunctionType
ALU = mybir.AluOpType
AX = mybir.AxisListType

B, NNODES, E, IN, H = 32, 1024, 262144, 8, 128
N = B * NNODES            # 32768
NCORES = 8
GPC = B // NCORES         # graphs per core = 4
NL = GPC * NNODES         # local nodes = 4096
NT = NL // 128            # dst tiles per core = 32
NGATHER = 8               # gather calls per layer
MSLOT = GPC * 128         # mask slots per core (128 per graph)


# ---------------------------------------------------------------- host prep
def prep(inputs):
    bf = ml_dtypes.bfloat16
    x = np.asarray(inputs['x'], np.float32)
    ei = np.asarray(inputs['edge_index'], np.int64)
    bv = np.asarray(inputs['batch_vec'], np.int64)
    mr = np.asarray(inputs['mask_rows'], np.int64)
    mc = np.asarray(inputs['mask_cols'], np.int64)

    src, dst = ei[0], ei[1]
    deg = np.bincount(dst, minlength=N).astype(np.float64)
    maxdeg = np.maximum(deg, 1.0)
    kappa = (deg > 0).astype(np.float32)
    wedge = (1.0 / maxdeg).astype(np.float32)   # weight per dst node

    meta = {}
    order = np.argsort(dst, kind='stable')
    s_sorted, d_sorted = src[order], dst[order]
    c_sorted = d_sorted // NL
    t_sorted = (d_sorted % NL) // 128
    counts = np.zeros((NCORES, NT), np.int64)
    for c in range(NCORES):
        m = c_sorted == c
        counts[c] = np.bincount(t_sorted[m], minlength=NT)
    CAP = int(np.ceil(max(1, counts.max()) / 128) * 128)
    CPT = CAP // 128
    NCHUNK = NT * CPT
    EPAD = NT * CAP
    TPG = NT // NGATHER
    GQ = TPG * CPT
    meta.update(CAP=CAP, CPT=CPT, NCHUNK=NCHUNK, EPAD=EPAD, TPG=TPG, GQ=GQ)

    gsrc = np.zeros((NCORES, 128, EPAD // 16), np.int16)
    dstloc = np.full((NCORES, 128, NCHUNK), -999.0, np.float32)
    vloc = np.zeros((NCORES, 128, NCHUNK), np.float32)
    oneirow = np.zeros((NCORES, 1, NL), np.float32)   # 1+kappa per local node
    for c in range(NCORES):
        m = c_sorted == c
        ss, tt = s_sorted[m], t_sorted[m]
        dd = (d_sorted[m] % NL) % 128
        ww = wedge[d_sorted[m]]
        idxs = np.zeros(EPAD, np.int64)
        dl = np.full(EPAD, -999.0, np.float32)
        vl = np.zeros(EPAD, np.float32)
        for t in range(NT):
            tm = tt == t
            k = int(tm.sum())
            base = t * CAP
            idxs[base:base + k] = ss[tm]
            dl[base:base + k] = dd[tm]
            vl[base:base + k] = ww[tm]
        per_call = TPG * CAP
        w = np.zeros((16, EPAD // 16), np.int16)
        for q in range(NGATHER):
            seg = idxs[q * per_call:(q + 1) * per_call]
            col0 = q * (per_call // 16)
            w[:, col0:col0 + per_call // 16] = seg.reshape(-1, 16).T
        gsrc[c] = np.tile(w, (8, 1))
        dstloc[c] = dl.reshape(NCHUNK, 128).T
        vloc[c] = vl.reshape(NCHUNK, 128).T
        oneirow[c, 0] = 1.0 + kappa[c * NL:(c + 1) * NL]

    # batch_vec per core (graph id values, tile-chunk layout)
    bvf = np.zeros((NCORES, 128, NT), np.float32)
    for c in range(NCORES):
        bvf[c] = bv[c * NL:(c + 1) * NL].astype(np.float32).reshape(NT, 128).T
    # host-side global graph sizes -> reciprocal counts for this core's graphs
    gcnt = np.maximum(np.bincount(bv, minlength=B), 1.0).astype(np.float32)
    rgcnt = np.zeros((NCORES, 128, 1), np.float32)
    for c in range(NCORES):
        rgcnt[c, :GPC, 0] = 1.0 / gcnt[c * GPC:(c + 1) * GPC]

    # mask entries (dedup) -> slots s = g_local*128 + k per core
    pair = mr * NNODES + mc
    keep = np.unique(pair)
    mrr = keep // NNODES
    mcc = keep % NNODES
    g_of = mrr // NNODES
    r_in_g = mrr % NNODES
    mridx = np.zeros((NCORES, 128, MSLOT // 16), np.int16)
    mcidx = np.zeros((NCORES, 128, MSLOT // 16), np.int16)
    mvalid = np.zeros((NCORES, 128, GPC), np.float32)
    # scatter tables split by row-occurrence so no two descriptors of one
    # call hit the same 512B row (concurrent scatter-add RMW would race)
    NSC = 1
    rows_all = np.zeros((NCORES, MSLOT), np.int64)
    cols_all = np.full((NCORES, MSLOT), -1, np.int64)
    for c in range(NCORES):
        ridx = np.zeros(MSLOT, np.int64)
        cidx = np.zeros(MSLOT, np.int64)
        for g in range(GPC):
            m = g_of == c * GPC + g
            k = int(m.sum())
            assert k <= 128, f"mask cap exceeded: {k}"
            rg = r_in_g[m]
            cg = mcc[m]
            pos = rg * NNODES + cg          # position within graph [0, 1M)
            ridx[g * 128:g * 128 + k] = g * NNODES + rg
            cidx[g * 128:g * 128 + k] = g * NNODES + cg
            rows_all[c, g * 128:g * 128 + k] = g * (NNODES * NNODES // 128) + pos // 128
            cols_all[c, g * 128:g * 128 + k] = pos % 128
            mvalid[c, :k, g] = 1.0
        mridx[c] = np.tile(ridx.reshape(-1, 16).T, (8, 1))
        mcidx[c] = np.tile(cidx.reshape(-1, 16).T, (8, 1))
        occ = np.zeros(MSLOT, np.int64)
        seen = {}
        for sslot in range(MSLOT):
            if cols_all[c, sslot] < 0:
                continue
            key = rows_all[c, sslot]
            occ[sslot] = seen.get(key, 0)
            seen[key] = occ[sslot] + 1
        NSC = max(NSC, int(occ.max()) + 1)
        rows_all[c] = np.where(cols_all[c] >= 0, rows_all[c], 0)
        if c == 0:
            occ_all = np.zeros((NCORES, MSLOT), np.int64)
        occ_all[c] = occ
    msrow = np.zeros((NCORES, NSC, 128, MSLOT // 16), np.int16)
    mcolmod = np.full((NCORES, NSC, 128, GPC), -999.0, np.float32)
    for c in range(NCORES):
        for sc in range(NSC):
            srow = np.zeros(MSLOT, np.int64)
            for sslot in range(MSLOT):
                if cols_all[c, sslot] >= 0 and occ_all[c, sslot] == sc:
                    srow[sslot] = rows_all[c, sslot]
                    mcolmod[c, sc, sslot % 128, sslot // 128] = cols_all[c, sslot]
            msrow[c, sc] = np.tile(srow.reshape(-1, 16).T, (8, 1))
    msrow = np.transpose(msrow, (0, 2, 1, 3)).reshape(NCORES, 128, NSC * (MSLOT // 16))
    mcolmod = np.transpose(mcolmod, (0, 2, 1, 3)).reshape(NCORES, 128, NSC * GPC)
    meta['NSC'] = NSC

    xT = np.stack([x[c * NL:(c + 1) * NL].T.copy() for c in range(NCORES)])

    # ---- f32 blob (weights/consts, shared across cores except dstloc/vloc/..)
    J128 = np.tile(np.arange(128, dtype=np.float32)[None, :], (128, 1))
    J32 = np.tile(np.arange(32, dtype=np.float32)[None, :], (128, 1))
    IDENT = np.eye(128, dtype=np.float32)
    ONESC = np.ones((128, 1), np.float32)

    def pad_pt(a, rows=128):
        out = np.zeros((rows, a.shape[1]), np.float32)
        out[:a.shape[0]] = a
        return out

    fcols = {}

    def fput(name, arr):
        fcols[name] = np.asarray(arr, np.float32)

    fput('IDENTf', IDENT)
    fput('J128f', J128)
    fput('onescf', ONESC)
    fput('onesrf', pad_pt(np.ones((1, 128), np.float32)))
    fput('b10', np.asarray(inputs['gin0_b1'], np.float32)[:, None])
    gb1 = np.asarray(inputs['gin_b1'], np.float32)
    for l in range(3):
        fput(f'gb1_{l}', gb1[l][:, None])
    bng = np.asarray(inputs['bn_gamma'], np.float32)
    bnb = np.asarray(inputs['bn_beta'], np.float32)
    for l in range(4):
        fput(f'bng_{l}', bng[l][:, None])
        fput(f'bnb_{l}', bnb[l][:, None])
    pw10 = np.asarray(inputs['pol0_w1'], np.float32)
    fput('pw10a', pw10[:128])
    fput('pw10b', pw10[128:])
    fput('pw20', np.asarray(inputs['pol0_w2'], np.float32))
    fput('pb10', np.asarray(inputs['pol0_b1'], np.float32)[:, None])
    fput('pb20', np.asarray(inputs['pol0_b2'], np.float32)[:, None])
    pw1 = np.asarray(inputs['pol_w1'], np.float32)
    pw2 = np.asarray(inputs['pol_w2'], np.float32)
    pb1 = np.asarray(inputs['pol_b1'], np.float32)
    pb2 = np.asarray(inputs['pol_b2'], np.float32)
    for l in range(2):
        fput(f'pw1_{l}', pw1[l])
        fput(f'pw2_{l}', pw2[l])
        fput(f'pb1_{l}', pb1[l][:, None])
        fput(f'pb2_{l}', pb2[l][:, None])
    # per-core pieces appended at fixed offsets
    fput('mcolmod', mcolmod[0])
    fput('mvalid', mvalid[0])
    fput('rgcnt', rgcnt[0])
    fput('dstloc', dstloc[0])
    fput('vloc', vloc[0])

    foffs = {}
    off = 0
    for name, a in fcols.items():
        foffs[name] = off
        off += a.shape[1]
    FC = off
    fblob = np.zeros((NCORES, 128, FC), np.float32)
    for c in range(NCORES):
        fcols['mcolmod'] = mcolmod[c]
        fcols['mvalid'] = mvalid[c]
        fcols['rgcnt'] = rgcnt[c]
        fcols['dstloc'] = dstloc[c]
        fcols['vloc'] = vloc[c]
        fblob[c] = np.concatenate(list(fcols.values()), axis=1)

    # ---- bf16 blob
    bcols = {}

    def bput(name, arr):
        bcols[name] = np.asarray(arr, np.float32).astype(bf)

    bput('j128b', J128)
    bput('identb', IDENT)
    bput('j32b', J32)
    bput('onescb', ONESC)
    bput('onesrb', pad_pt(np.ones((1, 128), np.float32)))
    bput('w10b', pad_pt(np.asarray(inputs['gin0_w1'], np.float32)))
    bput('w20b', np.asarray(inputs['gin0_w2'], np.float32))
    gw1 = np.asarray(inputs['gin_w1'], np.float32)
    gw2 = np.asarray(inputs['gin_w2'], np.float32)
    for l in range(3):
        bput(f'gw1_{l}', gw1[l])
        bput(f'gw2_{l}', gw2[l])
    b20 = np.asarray(inputs['gin0_b2'], np.float32)
    gb2 = np.asarray(inputs['gin_b2'], np.float32)
    bput('b20r', pad_pt(b20[None, :], rows=128))
    for l in range(3):
        bput(f'gb2r_{l}', pad_pt(gb2[l][None, :], rows=128))
    bput('pw10ab', pw10[:128])
    bput('pw20b', np.asarray(inputs['pol0_w2'], np.float32))
    for l in range(2):
        bput(f'pw1b_{l}', pw1[l])
        bput(f'pw2b_{l}', pw2[l])
    bput('bvfb', bvf[0])
    bput('oneirow', pad_pt(oneirow[0]))

    boffs = {}
    off = 0
    for name, a in bcols.items():
        boffs[name] = off
        off += a.shape[1]
    BC = off
    bblob = np.zeros((NCORES, 128, BC), bf)
    for c in range(NCORES):
        bcols['bvfb'] = bvf[c].astype(bf)
        bcols['oneirow'] = pad_pt(oneirow[c]).astype(bf)
        bblob[c] = np.concatenate([np.asarray(v) for v in bcols.values()], axis=1)

    meta['foffs'] = foffs
    meta['boffs'] = boffs
    meta['FC'] = FC
    meta['BC'] = BC

    in_maps = []
    for c in range(NCORES):
        im = dict(
            xT=xT[c].astype(bf), gsrc=gsrc[c],
            mridx=mridx[c], mcidx=mcidx[c], msrow=msrow[c],
            fblob=fblob[c], bblob=bblob[c],
        )
        in_maps.append(im)
    return in_maps, meta


# ---------------------------------------------------------------- builder
def build(meta, dbg=False, stage=99, shared_ag=True, no_cc=False):
    CAP, CPT, NCHUNK = meta['CAP'], meta['CPT'], meta['NCHUNK']
    EPAD, TPG, GQ = meta['EPAD'], meta['TPG'], meta['GQ']
    foffs, boffs = meta['foffs'], meta['boffs']
    FC, BC = meta['FC'], meta['BC']

    nc = bacc.Bacc(None, target_bir_lowering=False, debug=False)
    p_xT = nc.declare_dram_parameter("xT", [IN, NL], BF16, isOutput=False)
    p_gsrc = nc.declare_dram_parameter("gsrc", [128, EPAD // 16], I16, isOutput=False)
    p_mridx = nc.declare_dram_parameter("mridx", [128, MSLOT // 16], I16, isOutput=False)
    p_mcidx = nc.declare_dram_parameter("mcidx", [128, MSLOT // 16], I16, isOutput=False)
    NSC = meta["NSC"]
    p_msrow = nc.declare_dram_parameter("msrow", [128, NSC * (MSLOT // 16)], I16, isOutput=False)
    p_fblob = nc.declare_dram_parameter("fblob", [128, FC], F32, isOutput=False)
    p_bblob = nc.declare_dram_parameter("bblob", [128, BC], BF16, isOutput=False)
    p_pi = nc.declare_dram_parameter("pi", [GPC, NNODES * NNODES], F32, isOutput=True)
    p_dbg = nc.declare_dram_parameter("dbg", [8, 128, NL], F32, isOutput=True) if dbg else None

    d_hnode = nc.dram_tensor("d_hnode", [NL, H], BF16)
    d_hfull = nc.dram_tensor("d_hfull", [N, H], BF16, addr_space="Shared" if shared_ag else "Local")
    d_stat_in = nc.dram_tensor("d_stat_in", [128, 2], F32)
    d_stat_out = nc.dram_tensor("d_stat_out", [128, 2], F32, addr_space="Shared")
    d_z = nc.dram_tensor("d_z", [NL, H], F32)
    d_gp_in = nc.dram_tensor("d_gp_in", [32, H], F32)
    d_gp_out = nc.dram_tensor("d_gp_out", [GPC, H], F32)

    RG = [list(range(NCORES))]

    def CC(kind, op, ins, outs):
        if no_cc:
            n = min(ins[0].size(), outs[0].size())
            nc.sync.dma_start(outs[0].tensor.ap().rearrange("a b -> (a b)")[:n],
                              ins[0].tensor.ap().rearrange("a b -> (a b)")[:n])
        else:
            nc.gpsimd.collective_compute(kind, op, replica_groups=RG,
                                         ins=ins, outs=outs)

    with tile.TileContext(nc) as tc:
        pers = tc.alloc_tile_pool(name="pers", bufs=1)
        gp = tc.alloc_tile_pool(name="gp", bufs=2)
        wk = tc.alloc_tile_pool(name="wk", bufs=3)
        w1p = tc.alloc_tile_pool(name="w1p", bufs=1)
        psA = tc.alloc_tile_pool(name="psA", bufs=2, space="PSUM")
        psM = tc.alloc_tile_pool(name="psM", bufs=2, space="PSUM")
        psG = tc.alloc_tile_pool(name="psG", bufs=1, space="PSUM")
        psT = tc.alloc_tile_pool(name="psT", bufs=2, space="PSUM")

        # ---------- persistent SBUF ----------
        xT_sb0 = pers.tile([IN, NL], BF16, tag="xT")
        nc.sync.dma_start(xT_sb0[:], p_xT[:, :])
        bb = pers.tile([128, BC], BF16, tag="bb")
        nc.sync.dma_start(bb[:], p_bblob[:, :])
        gsrc_sb = pers.tile([128, EPAD // 16], I16, tag="gsrc")
        nc.sync.dma_start(gsrc_sb[:], p_gsrc[:, :])
        fb = pers.tile([128, FC], F32, tag="fb")
        nc.sync.dma_start(fb[:], p_fblob[:, :])

        def FB(name, ncol=1, rows=128, r0=0):
            c0 = foffs[name]
            return fb[r0:r0 + rows, c0:c0 + ncol]

        def BB(name, ncol=1, rows=128, r0=0):
            c0 = boffs[name]
            return bb[r0:r0 + rows, c0:c0 + ncol]

        IDENTf = FB('IDENTf', 128)
        J128f = FB('J128f', 128)
        onescf = FB('onescf', 1)
        onesrf = FB('onesrf', 128, rows=1)
        identb = BB('identb', 128)
        j128b = BB('j128b', 128)
        j32b = BB('j32b', 32)
        onescb = BB('onescb', 1)
        onesrb = BB('onesrb', 128, rows=1)
        dstloc = FB('dstloc', NCHUNK)
        vloc = FB('vloc', NCHUNK)

        zcol = pers.tile([128, 1], F32, tag="zcol", name="zcol")
        nc.vector.memset(zcol[:], 0.0)
        eps_t = pers.tile([128, 1], F32, tag="eps", name="eps_t")
        nc.vector.memset(eps_t[:], 1e-5)

        mridx_sb = pers.tile([128, MSLOT // 16], I16, tag="mridx")
        nc.sync.dma_start(mridx_sb[:], p_mridx[:, :])
        mcidx_sb = pers.tile([128, MSLOT // 16], I16, tag="mcidx")
        nc.sync.dma_start(mcidx_sb[:], p_mcidx[:, :])
        msrow_sb = pers.tile([128, NSC * (MSLOT // 16)], I16, tag="msrow")
        nc.sync.dma_start(msrow_sb[:], p_msrow[:, :])

        hnm = pers.tile([128, NL], BF16, tag="hnm")       # h' node-major
        np_fm = pers.tile([128, NL], F32, tag="npfm", name="np_fm")  # node_pool FM
        gp_accT = pers.tile([128, 32], F32, tag="gpacc")  # sum_l a.seg-sum(h') [f,g]
        oneir = BB('oneirow', NL, rows=1)                 # (1+kappa) row
        csum = pers.tile([128, 1], F32, tag="csum")       # sum of BN c vectors
        bvohc = pers.tile([128, NT * 32], BF16, tag="bvohc")

        def DUMP(i, t, cast=False):
            if dbg:
                if cast:
                    tmpd = pers.tile([128, NL], F32, tag="dmp", name=f"dmp{i}")
                    nc.vector.tensor_copy(tmpd[:], t[:, :NL])
                    nc.sync.dma_start(p_dbg.ap()[i], tmpd[:])
                else:
                    nc.sync.dma_start(p_dbg.ap()[i], t[:, :NL])

        # ---------- front: p node-major ----------
        xT_sb = xT_sb0
        for t in range(NT):
            sl = slice(t * 128, (t + 1) * 128)
            ps = psT.tile([128, 128], F32, tag="PT", name="pfront")
            nc.tensor.matmul(ps[:, :128], xT_sb[:IN, sl], BB('w10b', 128, rows=IN),
                             start=True, stop=True)
            nc.scalar.activation(hnm[:, sl], ps[:, :128], AF.Copy)

        d_hnode_v = d_hnode.ap().rearrange("(t p) f -> p t f", p=128)
        hnm_v = hnm[:].rearrange("p (t f) -> p t f", f=128)

        def write_hnode(q8):   # write tiles [8q, 8q+8)
            nc.sync.dma_start(d_hnode_v[:, q8 * 8:(q8 + 1) * 8, :],
                              hnm_v[:, q8 * 8:(q8 + 1) * 8, :])

        def ag():
            CC("AllGather", ALU.bypass, [d_hnode.ap().opt()], [d_hfull.ap().opt()])

        for q8 in range(4):
            write_hnode(q8)
        ag()
        DUMP(0, hnm, cast=True)

        # ---------- cached batch-vector one-hots [dst, 32] per tile ----------
        for t in range(NT):
            nc.vector.tensor_tensor(
                bvohc[:, t * 32:(t + 1) * 32],
                BB('bvfb', NT)[:, t:t + 1].to_broadcast([128, 32]),
                j32b, op=ALU.is_equal)

        # ---------- cached weighted one-hot ----------
        ohc = pers.tile([128, NCHUNK * 128], BF16, tag="ohc")
        for gc in range(NCHUNK):
            nc.vector.tensor_scalar(
                ohc[:, gc * 128:(gc + 1) * 128], j128b,
                dstloc[:, gc:gc + 1], vloc[:, gc:gc + 1],
                op0=ALU.is_equal, op1=ALU.mult)


        GCOLS16 = TPG * CAP // 16

        # per-layer BN-folded weights (built at runtime after each AR)
        w1s_cur = [None]
        cprow_cur = [None]

        # MLP weight names per layer (layer 0 has w10 pre-applied)
        w2names = ['w20b', 'gw2_0', 'gw2_1', 'gw2_2']
        b2names = ['b20r', 'gb2r_0', 'gb2r_1', 'gb2r_2']
        b1names = ['b10', 'gb1_0', 'gb1_1', 'gb1_2']

        for l in range(4):
            # ---- gathers of previous h' (or p) from d_hfull
            Gts = []
            for q in range(NGATHER):
                gt = gp.tile([128, GQ * 128], BF16, tag="G")
                nc.gpsimd.dma_gather(
                    gt[:].rearrange("p (c e) -> p c e", e=128),
                    d_hfull[:, :], gsrc_sb[:, q * GCOLS16:(q + 1) * GCOLS16],
                    TPG * CAP, TPG * CAP, 128, single_packet=False)
                Gts.append(gt)

            pgs = psG.tile([128, 132], F32, tag="PG", name=f"stats{l}")
            psg = pgs[:, 0:128]
            psm = pgs[:, 128:129]
            psseg = psG.tile([128, 32], F32, tag="SEG", name=f"seg{l}")
            for t in range(NT):
                sl = slice(t * 128, (t + 1) * 128)
                ps = psA.tile([128, 512], F32, tag="PP", name="psS")
                for c in range(CPT):
                    gc = t * CPT + c
                    q, lc = gc // GQ, gc % GQ
                    gch = Gts[q][:].rearrange("p (c e) -> p c e", e=128)[:, lc, :]
                    nc.tensor.matmul(ps[:, :128], gch,
                                     ohc[:, gc * 128:(gc + 1) * 128],
                                     start=(c == 0), stop=False)
                # + h'^T (fused transpose of node-major h)
                nc.tensor.matmul(ps[:, :128], hnm[:, sl], identb,
                                 start=False, stop=(l == 0))
                if l > 0:
                    # + c' x (1+kappa) rank-1
                    nc.tensor.matmul(ps[:, :128], cprow_cur[0],
                                     oneir[0:1, t * 128:(t + 1) * 128],
                                     start=False, stop=True)
                if l == 0:
                    rt = wk.tile([128, 128], BF16, tag="rt")
                    nc.scalar.activation(rt[:], ps[:, :128], AF.Relu, bias=FB('b10'))
                else:
                    s_sb = wk.tile([128, 128], BF16, tag="ssb")
                    nc.vector.tensor_copy(s_sb[:], ps[:, :128])
                    pu = psM.tile([128, 128], F32, tag="PM", name="pu")
                    nc.tensor.matmul(pu[:, :128], w1s_cur[0], s_sb[:],
                                     start=True, stop=True)
                    rt = wk.tile([128, 128], BF16, tag="rt")
                    nc.scalar.activation(rt[:], pu[:, :128], AF.Relu,
                                         bias=FB(b1names[l]))
                py = psM.tile([128, 128], F32, tag="PM", name="py")
                nc.tensor.matmul(py[:, :128], rt[:], BB(w2names[l], 128),
                                 start=True, stop=False)
                nc.tensor.matmul(py[:, :128], onesrb, BB(b2names[l], 128, rows=1),
                                 start=False, stop=True)
                nc.scalar.activation(hnm[:, sl], py[:, :128], AF.Relu, bias=zcol[:])
                # stats: gram (diag = sum sq) + sums
                nc.tensor.matmul(psg[:, :128], hnm[:, sl], hnm[:, sl],
                                 start=(t == 0), stop=(t == NT - 1))
                nc.tensor.matmul(psm[:, :], hnm[:, sl], onescb,
                                 start=(t == 0), stop=(t == NT - 1))
                nc.tensor.matmul(psseg[:, :32], hnm[:, sl],
                                 bvohc[:, t * 32:(t + 1) * 32],
                                 start=(t == 0), stop=(t == NT - 1))
                if l < 3 and t % 8 == 7:
                    write_hnode(t // 8)

            # ---- AllGather h' right away (pre-BN); stats AR runs in parallel
            if l < 3:
                ag()

            # ---- BN stats -> AllReduce -> affine constants
            dtile = wk.tile([128, 128], F32, tag="dtile")
            nc.vector.tensor_tensor(dtile[:], psg[:, :128], IDENTf, op=ALU.mult)
            stat = wk.tile([128, 2], F32, tag="stat")
            nc.vector.reduce_sum(stat[:, 1:2], dtile[:], axis=AX.X)
            nc.vector.tensor_copy(stat[:, 0:1], psm[:, :])
            nc.sync.dma_start(d_stat_in.ap(), stat[:])
            CC("AllReduce", ALU.add, [d_stat_in.ap().opt()], [d_stat_out.ap().opt()])
            gstat = wk.tile([128, 2], F32, tag="gstat")
            nc.sync.dma_start(gstat[:], d_stat_out.ap())
            mean = wk.tile([128, 1], F32, tag="mean")
            nc.vector.tensor_scalar_mul(mean[:], gstat[:, 0:1], 1.0 / N)
            var = wk.tile([128, 1], F32, tag="var")
            nc.vector.tensor_scalar_mul(var[:], gstat[:, 1:2], 1.0 / N)
            m2 = wk.tile([128, 1], F32, tag="m2")
            nc.vector.tensor_tensor(m2[:], mean[:], mean[:], op=ALU.mult)
            nc.vector.tensor_tensor(var[:], var[:], m2[:], op=ALU.subtract)
            std = wk.tile([128, 1], F32, tag="std")
            nc.scalar.activation(std[:], var[:], AF.Sqrt, bias=eps_t[:])
            rstd = wk.tile([128, 1], F32, tag="rstd")
            nc.vector.reciprocal(rstd[:], std[:])
            a_col = wk.tile([128, 1], F32, tag="acol")
            nc.vector.tensor_tensor(a_col[:], FB(f'bng_{l}'), rstd[:], op=ALU.mult)
            c_col = wk.tile([128, 1], F32, tag="ccol")
            nc.vector.tensor_tensor(c_col[:], mean[:], a_col[:], op=ALU.mult)
            nc.vector.tensor_tensor(c_col[:], FB(f'bnb_{l}'), c_col[:], op=ALU.subtract)
            # csum += c
            if l == 0:
                nc.vector.tensor_copy(csum[:], c_col[:])
            else:
                nc.vector.tensor_tensor(csum[:], csum[:], c_col[:], op=ALU.add)
            # gp_accT += a (.) seg-sum^T(h')
            if l == 0:
                nc.vector.tensor_scalar_mul(gp_accT[:], psseg[:, :32], a_col[:])
            else:
                nc.vector.scalar_tensor_tensor(gp_accT[:], psseg[:, :32], a_col[:],
                                               gp_accT[:], op0=ALU.mult, op1=ALU.add)
            # np_fm += a (.) h'^T  (c handled via csum folded into policy bias)
            for t in range(NT):
                sl = slice(t * 128, (t + 1) * 128)
                psf = psT.tile([128, 128], F32, tag="PT", name="npT")
                nc.tensor.matmul(psf[:, :128], hnm[:, sl], identb,
                                 start=True, stop=True)
                if l == 0:
                    nc.vector.tensor_scalar_mul(np_fm[:, sl], psf[:, :128], a_col[:])
                else:
                    nc.vector.scalar_tensor_tensor(np_fm[:, sl], psf[:, :128],
                                                   a_col[:], np_fm[:, sl],
                                                   op0=ALU.mult, op1=ALU.add)
            if l < 3:
                # w1' = diag(a) @ w1_{l+1}
                w1s = w1p.tile([128, 128], BF16, tag="w1s", name=f"w1s{l + 1}")
                nc.vector.tensor_scalar_mul(w1s[:], BB(f'gw1_{l}', 128), a_col[:])
                w1s_cur[0] = w1s[:]
                # c' = c / a as bf16 row
                ra = wk.tile([128, 1], F32, tag="ra")
                nc.vector.reciprocal(ra[:], a_col[:])
                cp = wk.tile([128, 1], F32, tag="cp")
                nc.vector.tensor_tensor(cp[:], c_col[:], ra[:], op=ALU.mult)
                cpb = wk.tile([128, 1], BF16, tag="cpb")
                nc.vector.tensor_copy(cpb[:], cp[:])
                psr3 = psT.tile([128, 128], F32, tag="PT", name=f"cprow{l}")
                nc.tensor.matmul(psr3[0:1, :128], cpb[:], identb, start=True, stop=True)
                cprow = w1p.tile([1, 128], BF16, tag="cprow", name=f"cpr{l + 1}")
                nc.vector.tensor_copy(cprow[:], psr3[0:1, :128])
                cprow_cur[0] = cprow[:]

        DUMP(1, hnm, cast=True)
        DUMP(2, np_fm, cast=True)
        if stage <= 2:
            for p in (psT, psG, psM, psA, w1p, wk, gp, pers):
                p.release()
            return nc

        # ---------------- gpool via ReduceScatter (gp_accT accumulated per layer)
        psq0 = psT.tile([128, 128], F32, tag="PT", name="gpinT")
        nc.tensor.transpose(psq0[:32, :128], gp_accT[:], IDENTf)
        gpin = wk.tile([32, H], F32, tag="gpin")
        nc.vector.tensor_copy(gpin[:, :H], psq0[:32, :128])
        nc.sync.dma_start(d_gp_in.ap(), gpin[:])
        CC("ReduceScatter", ALU.add, [d_gp_in.ap().opt()], [d_gp_out.ap().opt()])
        gpl = wk.tile([GPC, H], F32, tag="gpl")
        nc.sync.dma_start(gpl[:], d_gp_out.ap())
        gsc = wk.tile([GPC, H], F32, tag="gsc")
        nc.vector.tensor_scalar_mul(gsc[:], gpl[:, :H], FB('rgcnt', 1, rows=GPC))
        # gpT [f, g] = gsc^T via matmul with IDENT[:4,:4]
        psq = psT.tile([128, 128], F32, tag="PT", name="gpT")
        nc.tensor.matmul(psq[:, :GPC], gsc[:], IDENTf[:GPC, :GPC],
                         start=True, stop=True)
        gpT = wk.tile([128, GPC], F32, tag="gpT")
        nc.vector.tensor_copy(gpT[:], psq[:, :GPC])
        # gpool_true = segmean(np) + csum (BN c-terms deferred from np)
        nc.vector.tensor_tensor(gpT[:], gpT[:],
                                csum[:].to_broadcast([128, GPC]), op=ALU.add)
        # policy-0 per-graph bias cols: pb10 + w1a^T csum + w1b^T gpool_g
        psb = psT.tile([128, 128], F32, tag="PT", name="polbias")
        nc.tensor.matmul(psb[:, 0:GPC], FB('pw10b', 128), gpT[:],
                         start=True, stop=True)
        psc1 = psT.tile([128, 128], F32, tag="PT", name="polbias1")
        nc.tensor.matmul(psc1[:, 0:1], FB('pw10a', 128), csum[:],
                         start=True, stop=True)
        c1sb = wk.tile([128, 1], F32, tag="c1sb")
        nc.vector.tensor_copy(c1sb[:], psc1[:, 0:1])
        bias4 = pers.tile([128, GPC], F32, tag="bias4")
        nc.vector.tensor_tensor(bias4[:], psb[:, 0:GPC],
                                c1sb[:].to_broadcast([128, GPC]), op=ALU.add)
        nc.vector.tensor_scalar_add(bias4[:], bias4[:], FB('pb10'))

        # ---------------- pi zero-fill (DMA idle in the endgame region).
        # zft = np_fm * 0 gives the fill a dependency on the last GIN layer so
        # the scheduler cannot hoist this 47us DMA into the busy layer phase.
        zft = pers.tile([128, 512], F32, tag="zf")
        nc.vector.tensor_scalar_mul(zft[:], bias4[:, 0:1].to_broadcast([128, 512]), 0.0)
        nreps = (GPC * NNODES * NNODES) // (128 * 512)
        pi_fill_v = p_pi.ap().rearrange("g (a p e) -> (g a) p e", p=128, e=512
                                        ).rearrange("x p e -> p x e")
        nch = nreps // 8
        for fch in range(8):
            nc.gpsimd.dma_start(
                pi_fill_v[:, fch * nch:(fch + 1) * nch, :],
                zft[:].unsqueeze(1).to_broadcast([128, nch, 512]))


        # ---------------- policy MLP (feature-major, fp32r)
        # zfm aliases np_fm's buffer (np_fm fully consumed by the first pass)
        t1 = pers.tile([128, NL], BF16, tag="t1")

        def R(ap_):
            return ap_.bitcast(F32R)

        for j in range(NL // 512):
            js = slice(j * 512, (j + 1) * 512)
            g = j // 2
            ps1 = psA.tile([128, 512], F32, tag="PP", name="ps1")
            nc.tensor.matmul(ps1[:, :512], FB('pw10a', 128), np_fm[:, js],
                             start=True, stop=True)
            nc.scalar.activation(t1[:, js], ps1[:, :512], AF.Tanh,
                                 bias=bias4[:, g:g + 1])
        zfm = pers.tile([128, NL], BF16, tag="npfm", name="zfm")
        for j in range(NL // 512):
            js = slice(j * 512, (j + 1) * 512)
            ps2 = psA.tile([128, 512], F32, tag="PP", name="ps2")
            nc.tensor.matmul(ps2[:, :512], BB('pw20b', 128), t1[:, js],
                             start=True, stop=True)
            nc.vector.tensor_scalar_add(zfm[:, js], ps2[:, :512], FB('pb20'))
        znm = pers.tile([128, NL], F32, tag="znm", name="znm")
        d_z_v = d_z.ap().rearrange("(t p) f -> p t f", p=128)
        znm_v = znm[:].rearrange("p (t f) -> p t f", f=128)
        for j in range(NL // 512):
            js = slice(j * 512, (j + 1) * 512)
            ps1 = psA.tile([128, 512], F32, tag="PP", name="ps1")
            nc.tensor.matmul(ps1[:, :512], BB('pw1b_0', 128), zfm[:, js],
                             start=True, stop=True)
            nc.scalar.activation(t1[:, js], ps1[:, :512], AF.Tanh,
                                 bias=FB('pb1_0'))
        for j in range(NL // 512):
            js = slice(j * 512, (j + 1) * 512)
            ps2 = psA.tile([128, 512], F32, tag="PP", name="ps2")
            nc.tensor.matmul(ps2[:, :512], BB('pw2b_0', 128), t1[:, js],
                             start=True, stop=True)
            nc.vector.tensor_scalar_add(zfm[:, js], ps2[:, :512], FB('pb2_0'))
        for j in range(NL // 512):
            js = slice(j * 512, (j + 1) * 512)
            ps1 = psA.tile([128, 512], F32, tag="PP", name="ps1")
            nc.tensor.matmul(ps1[:, :512], BB('pw1b_1', 128), zfm[:, js],
                             start=True, stop=True)
            nc.scalar.activation(t1[:, js], ps1[:, :512], AF.Tanh,
                                 bias=FB('pb1_1'))
        zf32 = pers.tile([128, NL], F32, tag="zf32")
        for j in range(NL // 512):
            # final layer: z kept in f32 (feeds the softmax dots); transpose
            # tiles to node-major as soon as each chunk lands
            js = slice(j * 512, (j + 1) * 512)
            ps2 = psA.tile([128, 512], F32, tag="PP", name="ps2")
            nc.tensor.matmul(ps2[:, :512], BB('pw2b_1', 128), t1[:, js],
                             start=True, stop=True)
            nc.vector.tensor_scalar_add(zf32[:, js], ps2[:, :512], FB('pb2_1'))
            for t in range(4 * j, 4 * j + 4):
                sl = slice(t * 128, (t + 1) * 128)
                pst = psT.tile([128, 128], F32, tag="PT", name="zT")
                nc.tensor.transpose(pst[:, :128], zf32[:, sl], IDENTf)
                nc.vector.tensor_copy(znm[:, sl], pst[:, :128])
            nc.sync.dma_start(d_z_v[:, 4 * j:4 * j + 4, :],
                              znm_v[:, 4 * j:4 * j + 4, :])

        DUMP(4, zf32)
        if stage <= 4:
            for p in (psT, psG, psM, psA, w1p, wk, gp, pers):
                p.release()
            return nc

        # ---------------- masked sparse softmax
        mzr = pers.tile([128, MSLOT], F32, tag="mzr")
        nc.gpsimd.dma_gather(
            mzr[:].rearrange("p (c e) -> p c e", e=128),
            d_z[:, :], mridx_sb[:], MSLOT, MSLOT, 128, single_packet=False)
        mzc = pers.tile([128, MSLOT], F32, tag="mzc")
        nc.gpsimd.dma_gather(
            mzc[:].rearrange("p (c e) -> p c e", e=128),
            d_z[:, :], mcidx_sb[:], MSLOT, MSLOT, 128, single_packet=False)
        prod = pers.tile([128, MSLOT], F32, tag="pay", name="prod")
        nc.vector.tensor_tensor(prod[:], mzr[:], mzc[:], op=ALU.mult)
        dots = wk.tile([128, GPC], F32, tag="dots")
        nc.vector.reduce_sum(dots[:], prod[:].rearrange("p (g e) -> p g e", e=128),
                             axis=AX.X)
        em = wk.tile([128, GPC], F32, tag="em")
        nc.scalar.activation(em[:], dots[:], AF.Exp, bias=zcol[:])
        nc.vector.tensor_tensor(em[:], em[:], FB('mvalid', GPC), op=ALU.mult)
        psumg = psT.tile([128, 128], F32, tag="PT", name="psumg")
        nc.tensor.matmul(psumg[0:1, :GPC], onescf, em[:], start=True, stop=True)
        invT = wk.tile([1, GPC], F32, tag="invT")
        nc.vector.reciprocal(invT[:], psumg[0:1, :GPC])
        prep3 = psT.tile([128, 128], F32, tag="PT", name="prep3")
        nc.tensor.matmul(prep3[:, :GPC], onesrf, invT[:],
                         start=True, stop=True)
        vv = wk.tile([128, GPC], F32, tag="vv")
        nc.vector.tensor_tensor(vv[:], em[:], prep3[:, :GPC], op=ALU.mult)

        # scatter payload: slot (k, g) -> one-hot(col) * vv; one call per
        # row-occurrence level so no two descriptors of a call share a row
        for sc in range(NSC):
            pay = pers.tile([128, GPC * 128], F32, tag="pay", name=f"pay{sc}")
            for g in range(GPC):
                nc.vector.tensor_scalar(
                    pay[:, g * 128:(g + 1) * 128], J128f,
                    FB('mcolmod', NSC * GPC)[:, sc * GPC + g:sc * GPC + g + 1],
                    vv[:, g:g + 1], op0=ALU.is_equal, op1=ALU.mult)
            nc.gpsimd.dma_scatter_add(
                p_pi.ap().rearrange("g (r e) -> (g r) e", e=128),
                pay[:].rearrange("p (g e) -> p g e", e=128),
                msrow_sb[:, sc * (MSLOT // 16):(sc + 1) * (MSLOT // 16)],
                MSLOT, MSLOT, 128, single_packet=False)

        for p in (psT, psG, psM, psA, w1p, wk, gp, pers):
            p.release()
    return nc


# ---------------------------------------------------------------- numpy ref
def ref_np(inputs):
    x = np.asarray(inputs['x'], np.float64)
    ei = np.asarray(inputs['edge_index'])
    bv = np.asarray(inputs['batch_vec'])
    mr = np.asarray(inputs['mask_rows'])
    mc = np.asarray(inputs['mask_cols'])
    src, dst = ei[0], ei[1]
    deg = np.maximum(np.bincount(dst, minlength=N), 1.0)[:, None]

    def gin(h, w1, b1, w2, b2):
        agg = np.zeros_like(h)
        np.add.at(agg, dst, h[src])
        z = h + agg / deg
        return np.maximum(z @ w1 + b1, 0.0) @ w2 + b2

    def bn(h, g, b):
        mu = h.mean(0)
        var = h.var(0)
        return (h - mu) / np.sqrt(var + 1e-5) * g + b

    counts = np.maximum(np.bincount(bv, minlength=B), 1.0)[:, None]
    h = bn(np.maximum(gin(x, inputs['gin0_w1'], inputs['gin0_b1'],
                          inputs['gin0_w2'], inputs['gin0_b2']), 0.0),
           inputs['bn_gamma'][0], inputs['bn_beta'][0])
    node_pool = h.copy()
    gpool = np.zeros((B, H))
    np.add.at(gpool, bv, h)
    gpool = gpool / counts
    for l in range(3):
        h = bn(np.maximum(gin(h, inputs['gin_w1'][l], inputs['gin_b1'][l],
                              inputs['gin_w2'][l], inputs['gin_b2'][l]), 0.0),
               inputs['bn_gamma'][l + 1], inputs['bn_beta'][l + 1])
        node_pool += h
        gp2 = np.zeros((B, H))
        np.add.at(gp2, bv, h)
        gpool += gp2 / counts
    aug = np.concatenate([node_pool, np.repeat(gpool, NNODES, axis=0)],
                         axis=-1).reshape(B, NNODES, 2 * H)

    def pol(z, w1, b1, w2, b2):
        return np.tanh(z @ w1 + b1) @ w2 + b2

    z = pol(aug, inputs['pol0_w1'], inputs['pol0_b1'],
            inputs['pol0_w2'], inputs['pol0_b2'])
    for l in range(2):
        z = pol(z, inputs['pol_w1'][l], inputs['pol_b1'][l],
                inputs['pol_w2'][l], inputs['pol_b2'][l])
    score = np.einsum('bnd,bmd->bnm', z, z)
    mask = np.ones((B * NNODES, NNODES), bool)
    mask[mr, mc] = False
    score = np.where(mask.reshape(B, NNODES, NNODES), -np.inf, score)
    sc = score.reshape(B, -1)
    sc = sc - sc.max(-1, keepdims=True)
    e = np.exp(sc)
    return (e / e.sum(-1, keepdims=True)).astype(np.float32)


_CACHE = {}


def kernel(**inputs):
    in_maps, meta = prep(inputs)
    key = (meta['CAP'], meta['NSC'])
    if key not in _CACHE:
        nc = build(meta)
        nc.compile()
        _CACHE[key] = build_exec(nc, NCORES)
    run = _CACHE[key]
    res, times = run(in_maps, repeats=1)
    kernel.last_times = times
    return np.concatenate([res[c]["pi"].reshape(GPC, -1) for c in range(NCORES)], 0)


# revision 51
# speedup vs baseline: 1.0044x; 1.0044x over previous
"""GIN message-passing Actor network on 8 TRN2 NeuronCores (Bass/Tile).

Sharding: data-parallel over B=32 graphs -> 4 graphs (4096 nodes) per core.
Edges routed to the core owning dst, bucketed by 128-node dst tile.

v2 design:
- weighted one-hot (1/deg folded in) built ONCE via single tensor_scalar
  (is_equal+mult) ops, cached in SBUF bf16, reused across all 4 layers
- aggregation psum accumulates: edge-chunk matmuls (gathered bf16 rows
  stationary, cached one-hot moving) + h'^T via (h_nm stationary, IDENT
  moving) matmul (fuses the transpose) + rank-1 BN-offset correction
- BN deferred: AllGather h' (pre-BN) immediately after the MLP; BN stats
  AllReduce runs off critical path; the affine folds into the next layer's
  w1 (row scaling) and a rank-1 c-term; node_pool accumulates affine'd h'
- h' kept node-major (partition=node-in-tile): mm2 computed transposed
  (rt stationary, w2 moving), bias via rank-1; stats via Gram matmul
- all GIN matmuls bf16; policy MLP fp32r (full speed at 512-wide)
- output pi: broadcast-DMA zero-fill at t=0 + dma_scatter_add of the
  ~64 softmax nonzeros per graph (512B rows)
"""
import sys
sys.path.insert(0, '/opt/trn_rl_repo')
import numpy as np
import ml_dtypes
from concourse import bass, mybir, tile, bacc

import jax
import time
from jax.sharding import Mesh, PartitionSpec
from jax.experimental.shard_map import shard_map
from concourse import bass2jax
from concourse.bass2jax import _bass_exec_p, partition_id_tensor, install_neuronx_cc_hook


def build_exec(nc, n_cores, chain=1):
    install_neuronx_cc_hook()
    partition_name = nc.partition_id_tensor.name if nc.partition_id_tensor else None
    in_names, out_names, out_avals, zero_outs = [], [], [], []
    for alloc in nc.m.functions[0].allocations:
        if not isinstance(alloc, mybir.MemoryLocationSet):
            continue
        name = alloc.memorylocations[0].name
        if alloc.kind == "ExternalInput":
            if name != partition_name:
                in_names.append(name)
        elif alloc.kind == "ExternalOutput":
            shape = tuple(alloc.tensor_shape)
            dtype = mybir.dt.np(alloc.dtype)
            out_names.append(name)
            out_avals.append(jax.core.ShapedArray(shape, dtype))
            zero_outs.append(np.zeros(shape, dtype))
    n_params = len(in_names)
    all_in = list(in_names) + list(out_names)
    if partition_name is not None:
        all_in.append(partition_name)

    def _body(*args):
        operands = list(args)
        if partition_name is not None:
            operands.append(partition_id_tensor())
        for _ in range(chain):
            outs = _bass_exec_p.bind(
                *operands,
                out_avals=tuple(out_avals),
                in_names=tuple(all_in),
                out_names=tuple(out_names),
                lowering_input_output_aliases=(),
                sim_require_finite=True, sim_require_nnan=True, nc=nc,
            )
        return tuple(outs)

    devices = jax.devices()[:n_cores]
    mesh = Mesh(np.asarray(devices), ("core",))
    in_specs = (PartitionSpec("core"),) * (n_params + len(out_names))
    out_specs = (PartitionSpec("core"),) * len(out_names)
    sharded = jax.jit(shard_map(_body, mesh=mesh, in_specs=in_specs,
                                out_specs=out_specs, check_rep=False),
                      keep_unused=True)

    def run(in_maps, repeats=1):
        per_core = [[np.asarray(m[n]) for n in in_names] for m in in_maps]
        concat_in = [np.concatenate([per_core[c][i] for c in range(n_cores)], axis=0)
                     for i in range(n_params)]
        concat_zeros = [np.zeros((n_cores * z.shape[0], *z.shape[1:]), z.dtype) for z in zero_outs]
        times = []
        out_arrs = None
        for r in range(repeats):
            t0 = time.perf_counter()
            out_arrs = sharded(*concat_in, *concat_zeros)
            jax.block_until_ready(out_arrs)
            times.append(time.perf_counter() - t0)
        results = [
            {name: np.asarray(out_arrs[i]).reshape(n_cores, *out_avals[i].shape)[c]
             for i, name in enumerate(out_names)}
            for c in range(n_cores)
        ]
        return results, times
    return run


F32 = mybir.dt.float32
F32R = mybir.dt.float32r
BF16 = mybir.dt.bfloat16
I16 = mybir.dt.int16
AF = mybir.ActivationFunctionType
ALU = mybir.AluOpType
AX = mybir.AxisListType

B, NNODES, E, IN, H = 32, 1024, 262144, 8, 128
N = B * NNODES            # 32768
NCORES = 8
GPC = B // NCORES         # graphs per core = 4
NL = GPC * NNODES         # local nodes = 4096
NT = NL // 128            # dst tiles per core = 32
NGATHER = 8               # gather calls per layer
MSLOT = GPC * 128         # mask slots per core (128 per graph)


# ---------------------------------------------------------------- host prep
def prep(inputs):
    bf = ml_dtypes.bfloat16
    x = np.asarray(inputs['x'], np.float32)
    ei = np.asarray(inputs['edge_index'], np.int64)
    bv = np.asarray(inputs['batch_vec'], np.int64)
    mr = np.asarray(inputs['mask_rows'], np.int64)
    mc = np.asarray(inputs['mask_cols'], np.int64)

    src, dst = ei[0], ei[1]
    deg = np.bincount(dst, minlength=N).astype(np.float64)
    maxdeg = np.maximum(deg, 1.0)
    kappa = (deg > 0).astype(np.float32)
    wedge = (1.0 / maxdeg).astype(np.float32)   # weight per dst node

    meta = {}
    order = np.argsort(dst, kind='stable')
    s_sorted, d_sorted = src[order], dst[order]
    c_sorted = d_sorted // NL
    t_sorted = (d_sorted % NL) // 128
    counts = np.zeros((NCORES, NT), np.int64)
    for c in range(NCORES):
        m = c_sorted == c
        counts[c] = np.bincount(t_sorted[m], minlength=NT)
    CAP = int(np.ceil(max(1, counts.max()) / 128) * 128)
    CPT = CAP // 128
    NCHUNK = NT * CPT
    EPAD = NT * CAP
    TPG = NT // NGATHER
    GQ = TPG * CPT
    meta.update(CAP=CAP, CPT=CPT, NCHUNK=NCHUNK, EPAD=EPAD, TPG=TPG, GQ=GQ)

    gsrc = np.zeros((NCORES, 128, EPAD // 16), np.int16)
    dstloc = np.full((NCORES, 128, NCHUNK), -999.0, np.float32)
    vloc = np.zeros((NCORES, 128, NCHUNK), np.float32)
    oneirow = np.zeros((NCORES, 1, NL), np.float32)   # 1+kappa per local node
    for c in range(NCORES):
        m = c_sorted == c
        ss, tt = s_sorted[m], t_sorted[m]
        dd = (d_sorted[m] % NL) % 128
        ww = wedge[d_sorted[m]]
        idxs = np.zeros(EPAD, np.int64)
        dl = np.full(EPAD, -999.0, np.float32)
        vl = np.zeros(EPAD, np.float32)
        for t in range(NT):
            tm = tt == t
            k = int(tm.sum())
            base = t * CAP
            idxs[base:base + k] = ss[tm]
            dl[base:base + k] = dd[tm]
            vl[base:base + k] = ww[tm]
        per_call = TPG * CAP
        w = np.zeros((16, EPAD // 16), np.int16)
        for q in range(NGATHER):
            seg = idxs[q * per_call:(q + 1) * per_call]
            col0 = q * (per_call // 16)
            w[:, col0:col0 + per_call // 16] = seg.reshape(-1, 16).T
        gsrc[c] = np.tile(w, (8, 1))
        dstloc[c] = dl.reshape(NCHUNK, 128).T
        vloc[c] = vl.reshape(NCHUNK, 128).T
        oneirow[c, 0] = 1.0 + kappa[c * NL:(c + 1) * NL]

    # batch_vec per core (graph id values, tile-chunk layout)
    bvf = np.zeros((NCORES, 128, NT), np.float32)
    for c in range(NCORES):
        bvf[c] = bv[c * NL:(c + 1) * NL].astype(np.float32).reshape(NT, 128).T
    # host-side global graph sizes -> reciprocal counts for this core's graphs
    gcnt = np.maximum(np.bincount(bv, minlength=B), 1.0).astype(np.float32)
    rgcnt = np.zeros((NCORES, 128, 1), np.float32)
    for c in range(NCORES):
        rgcnt[c, :GPC, 0] = 1.0 / gcnt[c * GPC:(c + 1) * GPC]

    # mask entries (dedup) -> slots s = g_local*128 + k per core
    pair = mr * NNODES + mc
    keep = np.unique(pair)
    mrr = keep // NNODES
    mcc = keep % NNODES
    g_of = mrr // NNODES
    r_in_g = mrr % NNODES
    mridx = np.zeros((NCORES, 128, MSLOT // 16), np.int16)
    mcidx = np.zeros((NCORES, 128, MSLOT // 16), np.int16)
    mvalid = np.zeros((NCORES, 128, GPC), np.float32)
    # scatter tables split by row-occurrence so no two descriptors of one
    # call hit the same 512B row (concurrent scatter-add RMW would race)
    NSC = 1
    rows_all = np.zeros((NCORES, MSLOT), np.int64)
    cols_all = np.full((NCORES, MSLOT), -1, np.int64)
    for c in range(NCORES):
        ridx = np.zeros(MSLOT, np.int64)
        cidx = np.zeros(MSLOT, np.int64)
        for g in range(GPC):
            m = g_of == c * GPC + g
            k = int(m.sum())
            assert k <= 128, f"mask cap exceeded: {k}"
            rg = r_in_g[m]
            cg = mcc[m]
            pos = rg * NNODES + cg          # position within graph [0, 1M)
            ridx[g * 128:g * 128 + k] = g * NNODES + rg
            cidx[g * 128:g * 128 + k] = g * NNODES + cg
            rows_all[c, g * 128:g * 128 + k] = g * (NNODES * NNODES // 128) + pos // 128
            cols_all[c, g * 128:g * 128 + k] = pos % 128
            mvalid[c, :k, g] = 1.0
        mridx[c] = np.tile(ridx.reshape(-1, 16).T, (8, 1))
        mcidx[c] = np.tile(cidx.reshape(-1, 16).T, (8, 1))
        occ = np.zeros(MSLOT, np.int64)
        seen = {}
        for sslot in range(MSLOT):
            if cols_all[c, sslot] < 0:
                continue
            key = rows_all[c, sslot]
            occ[sslot] = seen.get(key, 0)
            seen[key] = occ[sslot] + 1
        NSC = max(NSC, int(occ.max()) + 1)
        rows_all[c] = np.where(cols_all[c] >= 0, rows_all[c], 0)
        if c == 0:
            occ_all = np.zeros((NCORES, MSLOT), np.int64)
        occ_all[c] = occ
    msrow = np.zeros((NCORES, NSC, 128, MSLOT // 16), np.int16)
    mcolmod = np.full((NCORES, NSC, 128, GPC), -999.0, np.float32)
    for c in range(NCORES):
        for sc in range(NSC):
            srow = np.zeros(MSLOT, np.int64)
            for sslot in range(MSLOT):
                if cols_all[c, sslot] >= 0 and occ_all[c, sslot] == sc:
                    srow[sslot] = rows_all[c, sslot]
                    mcolmod[c, sc, sslot % 128, sslot // 128] = cols_all[c, sslot]
            msrow[c, sc] = np.tile(srow.reshape(-1, 16).T, (8, 1))
    msrow = np.transpose(msrow, (0, 2, 1, 3)).reshape(NCORES, 128, NSC * (MSLOT // 16))
    mcolmod = np.transpose(mcolmod, (0, 2, 1, 3)).reshape(NCORES, 128, NSC * GPC)
    meta['NSC'] = NSC

    xT = np.stack([x[c * NL:(c + 1) * NL].T.copy() for c in range(NCORES)])

    # ---- f32 blob (weights/consts, shared across cores except dstloc/vloc/..)
    J128 = np.tile(np.arange(128, dtype=np.float32)[None, :], (128, 1))
    J32 = np.tile(np.arange(32, dtype=np.float32)[None, :], (128, 1))
    IDENT = np.eye(128, dtype=np.float32)
    ONESC = np.ones((128, 1), np.float32)

    def pad_pt(a, rows=128):
        out = np.zeros((rows, a.shape[1]), np.float32)
        out[:a.shape[0]] = a
        return out

    fcols = {}

    def fput(name, arr):
        fcols[name] = np.asarray(arr, np.float32)

    fput('IDENTf', IDENT)
    fput('J128f', J128)
    fput('onescf', ONESC)
    fput('onesrf', pad_pt(np.ones((1, 128), np.float32)))
    fput('b10', np.asarray(inputs['gin0_b1'], np.float32)[:, None])
    gb1 = np.asarray(inputs['gin_b1'], np.float32)
    for l in range(3):
        fput(f'gb1_{l}', gb1[l][:, None])
    bng = np.asarray(inputs['bn_gamma'], np.float32)
    bnb = np.asarray(inputs['bn_beta'], np.float32)
    for l in range(4):
        fput(f'bng_{l}', bng[l][:, None])
        fput(f'bnb_{l}', bnb[l][:, None])
    pw10 = np.asarray(inputs['pol0_w1'], np.float32)
    fput('pw10a', pw10[:128])
    fput('pw10b', pw10[128:])
    fput('pw20', np.asarray(inputs['pol0_w2'], np.float32))
    fput('pb10', np.asarray(inputs['pol0_b1'], np.float32)[:, None])
    fput('pb20', np.asarray(inputs['pol0_b2'], np.float32)[:, None])
    pw1 = np.asarray(inputs['pol_w1'], np.float32)
    pw2 = np.asarray(inputs['pol_w2'], np.float32)
    pb1 = np.asarray(inputs['pol_b1'], np.float32)
    pb2 = np.asarray(inputs['pol_b2'], np.float32)
    for l in range(2):
        fput(f'pw1_{l}', pw1[l])
        fput(f'pw2_{l}', pw2[l])
        fput(f'pb1_{l}', pb1[l][:, None])
        fput(f'pb2_{l}', pb2[l][:, None])
    # per-core pieces appended at fixed offsets
    fput('mcolmod', mcolmod[0])
    fput('mvalid', mvalid[0])
    fput('rgcnt', rgcnt[0])
    fput('dstloc', dstloc[0])
    fput('vloc', vloc[0])

    foffs = {}
    off = 0
    for name, a in fcols.items():
        foffs[name] = off
        off += a.shape[1]
    FC = off
    fblob = np.zeros((NCORES, 128, FC), np.float32)
    for c in range(NCORES):
        fcols['mcolmod'] = mcolmod[c]
        fcols['mvalid'] = mvalid[c]
        fcols['rgcnt'] = rgcnt[c]
        fcols['dstloc'] = dstloc[c]
        fcols['vloc'] = vloc[c]
        fblob[c] = np.concatenate(list(fcols.values()), axis=1)

    # ---- bf16 blob
    bcols = {}

    def bput(name, arr):
        bcols[name] = np.asarray(arr, np.float32).astype(bf)

    bput('j128b', J128)
    bput('identb', IDENT)
    bput('j32b', J32)
    bput('onescb', ONESC)
    bput('onesrb', pad_pt(np.ones((1, 128), np.float32)))
    bput('w10b', pad_pt(np.asarray(inputs['gin0_w1'], np.float32)))
    bput('w20b', np.asarray(inputs['gin0_w2'], np.float32))
    gw1 = np.asarray(inputs['gin_w1'], np.float32)
    gw2 = np.asarray(inputs['gin_w2'], np.float32)
    for l in range(3):
        bput(f'gw1_{l}', gw1[l])
        bput(f'gw2_{l}', gw2[l])
    b20 = np.asarray(inputs['gin0_b2'], np.float32)
    gb2 = np.asarray(inputs['gin_b2'], np.float32)
    bput('b20r', pad_pt(b20[None, :], rows=128))
    for l in range(3):
        bput(f'gb2r_{l}', pad_pt(gb2[l][None, :], rows=128))
    bput('pw10ab', pw10[:128])
    bput('pw20b', np.asarray(inputs['pol0_w2'], np.float32))
    for l in range(2):
        bput(f'pw1b_{l}', pw1[l])
        bput(f'pw2b_{l}', pw2[l])
    bput('bvfb', bvf[0])
    bput('oneirow', pad_pt(oneirow[0]))

    boffs = {}
    off = 0
    for name, a in bcols.items():
        boffs[name] = off
        off += a.shape[1]
    BC = off
    bblob = np.zeros((NCORES, 128, BC), bf)
    for c in range(NCORES):
        bcols['bvfb'] = bvf[c].astype(bf)
        bcols['oneirow'] = pad_pt(oneirow[c]).astype(bf)
        bblob[c] = np.concatenate([np.asarray(v) for v in bcols.values()], axis=1)

    meta['foffs'] = foffs
    meta['boffs'] = boffs
    meta['FC'] = FC
    meta['BC'] = BC

    in_maps = []
    for c in range(NCORES):
        im = dict(
            xT=xT[c].astype(bf), gsrc=gsrc[c],
            mridx=mridx[c], mcidx=mcidx[c], msrow=msrow[c],
            fblob=fblob[c], bblob=bblob[c],
        )
        in_maps.append(im)
    return in_maps, meta


# ---------------------------------------------------------------- builder
def build(meta, dbg=False, stage=99, shared_ag=True, no_cc=False):
    CAP, CPT, NCHUNK = meta['CAP'], meta['CPT'], meta['NCHUNK']
    EPAD, TPG, GQ = meta['EPAD'], meta['TPG'], meta['GQ']
    foffs, boffs = meta['foffs'], meta['boffs']
    FC, BC = meta['FC'], meta['BC']

    nc = bacc.Bacc(None, target_bir_lowering=False, debug=False)
    p_xT = nc.declare_dram_parameter("xT", [IN, NL], BF16, isOutput=False)
    p_gsrc = nc.declare_dram_parameter("gsrc", [128, EPAD // 16], I16, isOutput=False)
    p_mridx = nc.declare_dram_parameter("mridx", [128, MSLOT // 16], I16, isOutput=False)
    p_mcidx = nc.declare_dram_parameter("mcidx", [128, MSLOT // 16], I16, isOutput=False)
    NSC = meta["NSC"]
    p_msrow = nc.declare_dram_parameter("msrow", [128, NSC * (MSLOT // 16)], I16, isOutput=False)
    p_fblob = nc.declare_dram_parameter("fblob", [128, FC], F32, isOutput=False)
    p_bblob = nc.declare_dram_parameter("bblob", [128, BC], BF16, isOutput=False)
    p_pi = nc.declare_dram_parameter("pi", [GPC, NNODES * NNODES], F32, isOutput=True)
    p_dbg = nc.declare_dram_parameter("dbg", [8, 128, NL], F32, isOutput=True) if dbg else None

    d_hnode = nc.dram_tensor("d_hnode", [NL, H], BF16)
    d_hfull = nc.dram_tensor("d_hfull", [N, H], BF16, addr_space="Shared" if shared_ag else "Local")
    d_stat_in = nc.dram_tensor("d_stat_in", [128, 2], F32)
    d_stat_out = nc.dram_tensor("d_stat_out", [128, 2], F32, addr_space="Shared")
    d_z = nc.dram_tensor("d_z", [NL, H], F32)
    d_gp_in = nc.dram_tensor("d_gp_in", [32, H], F32)
    d_gp_out = nc.dram_tensor("d_gp_out", [GPC, H], F32)

    RG = [list(range(NCORES))]

    def CC(kind, op, ins, outs):
        if no_cc:
            n = min(ins[0].size(), outs[0].size())
            nc.sync.dma_start(outs[0].tensor.ap().rearrange("a b -> (a b)")[:n],
                              ins[0].tensor.ap().rearrange("a b -> (a b)")[:n])
        else:
            nc.gpsimd.collective_compute(kind, op, replica_groups=RG,
                                         ins=ins, outs=outs)

    with tile.TileContext(nc) as tc:
        pers = tc.alloc_tile_pool(name="pers", bufs=1)
        gp = tc.alloc_tile_pool(name="gp", bufs=2)
        wk = tc.alloc_tile_pool(name="wk", bufs=3)
        w1p = tc.alloc_tile_pool(name="w1p", bufs=1)
        psA = tc.alloc_tile_pool(name="psA", bufs=2, space="PSUM")
        psM = tc.alloc_tile_pool(name="psM", bufs=2, space="PSUM")
        psG = tc.alloc_tile_pool(name="psG", bufs=1, space="PSUM")
        psT = tc.alloc_tile_pool(name="psT", bufs=2, space="PSUM")

        # ---------- persistent SBUF ----------
        xT_sb0 = pers.tile([IN, NL], BF16, tag="xT")
        nc.sync.dma_start(xT_sb0[:], p_xT[:, :])
        bb = pers.tile([128, BC], BF16, tag="bb")
        nc.sync.dma_start(bb[:], p_bblob[:, :])
        gsrc_sb = pers.tile([128, EPAD // 16], I16, tag="gsrc")
        nc.sync.dma_start(gsrc_sb[:], p_gsrc[:, :])
        fb = pers.tile([128, FC], F32, tag="fb")
        nc.sync.dma_start(fb[:], p_fblob[:, :])

        def FB(name, ncol=1, rows=128, r0=0):
            c0 = foffs[name]
            return fb[r0:r0 + rows, c0:c0 + ncol]

        def BB(name, ncol=1, rows=128, r0=0):
            c0 = boffs[name]
            return bb[r0:r0 + rows, c0:c0 + ncol]

        IDENTf = FB('IDENTf', 128)
        J128f = FB('J128f', 128)
        onescf = FB('onescf', 1)
        onesrf = FB('onesrf', 128, rows=1)
        identb = BB('identb', 128)
        j128b = BB('j128b', 128)
        j32b = BB('j32b', 32)
        onescb = BB('onescb', 1)
        onesrb = BB('onesrb', 128, rows=1)
        dstloc = FB('dstloc', NCHUNK)
        vloc = FB('vloc', NCHUNK)

        zcol = pers.tile([128, 1], F32, tag="zcol", name="zcol")
        nc.vector.memset(zcol[:], 0.0)
        eps_t = pers.tile([128, 1], F32, tag="eps", name="eps_t")
        nc.vector.memset(eps_t[:], 1e-5)

        mridx_sb = pers.tile([128, MSLOT // 16], I16, tag="mridx")
        nc.sync.dma_start(mridx_sb[:], p_mridx[:, :])
        mcidx_sb = pers.tile([128, MSLOT // 16], I16, tag="mcidx")
        nc.sync.dma_start(mcidx_sb[:], p_mcidx[:, :])
        msrow_sb = pers.tile([128, NSC * (MSLOT // 16)], I16, tag="msrow")
        nc.sync.dma_start(msrow_sb[:], p_msrow[:, :])

        hnm = pers.tile([128, NL], BF16, tag="hnm")       # h' node-major
        np_fm = pers.tile([128, NL], BF16, tag="npfm", name="np_fm")  # node_pool FM
        gp_accT = pers.tile([128, 32], F32, tag="gpacc")  # sum_l a.seg-sum(h') [f,g]
        oneir = BB('oneirow', NL, rows=1)                 # (1+kappa) row
        csum = pers.tile([128, 1], F32, tag="csum")       # sum of BN c vectors
        bvohc = pers.tile([128, NT * 32], BF16, tag="bvohc")

        def DUMP(i, t, cast=False):
            if dbg:
                if cast:
                    tmpd = pers.tile([128, NL], F32, tag="dmp", name=f"dmp{i}")
                    nc.vector.tensor_copy(tmpd[:], t[:, :NL])
                    nc.sync.dma_start(p_dbg.ap()[i], tmpd[:])
                else:
                    nc.sync.dma_start(p_dbg.ap()[i], t[:, :NL])

        # ---------- front: p node-major ----------
        xT_sb = xT_sb0
        for t in range(NT):
            sl = slice(t * 128, (t + 1) * 128)
            ps = psT.tile([128, 128], F32, tag="PT", name="pfront")
            nc.tensor.matmul(ps[:, :128], xT_sb[:IN, sl], BB('w10b', 128, rows=IN),
                             start=True, stop=True)
            nc.scalar.activation(hnm[:, sl], ps[:, :128], AF.Copy)

        d_hnode_v = d_hnode.ap().rearrange("(t p) f -> p t f", p=128)
        hnm_v = hnm[:].rearrange("p (t f) -> p t f", f=128)

        def write_hnode(q8):   # write tiles [8q, 8q+8)
            nc.sync.dma_start(d_hnode_v[:, q8 * 8:(q8 + 1) * 8, :],
                              hnm_v[:, q8 * 8:(q8 + 1) * 8, :])

        def ag():
            CC("AllGather", ALU.bypass, [d_hnode.ap().opt()], [d_hfull.ap().opt()])

        for q8 in range(4):
            write_hnode(q8)
        ag()
        DUMP(0, hnm, cast=True)

        # ---------- cached batch-vector one-hots [dst, 32] per tile ----------
        for t in range(NT):
            nc.vector.tensor_tensor(
                bvohc[:, t * 32:(t + 1) * 32],
                BB('bvfb', NT)[:, t:t + 1].to_broadcast([128, 32]),
                j32b, op=ALU.is_equal)

        # ---------- cached weighted one-hot ----------
        ohc = pers.tile([128, NCHUNK * 128], BF16, tag="ohc")
        for gc in range(NCHUNK):
            nc.vector.tensor_scalar(
                ohc[:, gc * 128:(gc + 1) * 128], j128b,
                dstloc[:, gc:gc + 1], vloc[:, gc:gc + 1],
                op0=ALU.is_equal, op1=ALU.mult)


        GCOLS16 = TPG * CAP // 16

        # per-layer BN-folded weights (built at runtime after each AR)
        w1s_cur = [None]
        cprow_cur = [None]

        # MLP weight names per layer (layer 0 has w10 pre-applied)
        w2names = ['w20b', 'gw2_0', 'gw2_1', 'gw2_2']
        b2names = ['b20r', 'gb2r_0', 'gb2r_1', 'gb2r_2']
        b1names = ['b10', 'gb1_0', 'gb1_1', 'gb1_2']

        for l in range(4):
            # ---- gathers of previous h' (or p) from d_hfull
            Gts = []
            for q in range(NGATHER):
                gt = gp.tile([128, GQ * 128], BF16, tag="G")
                nc.gpsimd.dma_gather(
                    gt[:].rearrange("p (c e) -> p c e", e=128),
                    d_hfull[:, :], gsrc_sb[:, q * GCOLS16:(q + 1) * GCOLS16],
                    TPG * CAP, TPG * CAP, 128, single_packet=False)
                Gts.append(gt)

            pgs = psG.tile([128, 132], F32, tag="PG", name=f"stats{l}")
            psg = pgs[:, 0:128]
            psm = pgs[:, 128:129]
            psseg = psG.tile([128, 32], F32, tag="SEG", name=f"seg{l}")
            for t in range(NT):
                sl = slice(t * 128, (t + 1) * 128)
                ps = psA.tile([128, 512], F32, tag="PP", name="psS")
                for c in range(CPT):
                    gc = t * CPT + c
                    q, lc = gc // GQ, gc % GQ
                    gch = Gts[q][:].rearrange("p (c e) -> p c e", e=128)[:, lc, :]
                    nc.tensor.matmul(ps[:, :128], gch,
                                     ohc[:, gc * 128:(gc + 1) * 128],
                                     start=(c == 0), stop=False)
                # + h'^T (fused transpose of node-major h)
                nc.tensor.matmul(ps[:, :128], hnm[:, sl], identb,
                                 start=False, stop=(l == 0))
                if l > 0:
                    # + c' x (1+kappa) rank-1
                    nc.tensor.matmul(ps[:, :128], cprow_cur[0],
                                     oneir[0:1, t * 128:(t + 1) * 128],
                                     start=False, stop=True)
                if l == 0:
                    rt = wk.tile([128, 128], BF16, tag="rt")
                    nc.scalar.activation(rt[:], ps[:, :128], AF.Relu, bias=FB('b10'))
                else:
                    s_sb = wk.tile([128, 128], BF16, tag="ssb")
                    nc.vector.tensor_copy(s_sb[:], ps[:, :128])
                    pu = psM.tile([128, 128], F32, tag="PM", name="pu")
                    nc.tensor.matmul(pu[:, :128], w1s_cur[0], s_sb[:],
                                     start=True, stop=True)
                    rt = wk.tile([128, 128], BF16, tag="rt")
                    nc.scalar.activation(rt[:], pu[:, :128], AF.Relu,
                                         bias=FB(b1names[l]))
                py = psM.tile([128, 128], F32, tag="PM", name="py")
                nc.tensor.matmul(py[:, :128], rt[:], BB(w2names[l], 128),
                                 start=True, stop=False)
                nc.tensor.matmul(py[:, :128], onesrb, BB(b2names[l], 128, rows=1),
                                 start=False, stop=True)
                nc.scalar.activation(hnm[:, sl], py[:, :128], AF.Relu, bias=zcol[:])
                # stats: gram (diag = sum sq) + sums
                nc.tensor.matmul(psg[:, :128], hnm[:, sl], hnm[:, sl],
                                 start=(t == 0), stop=(t == NT - 1))
                nc.tensor.matmul(psm[:, :], hnm[:, sl], onescb,
                                 start=(t == 0), stop=(t == NT - 1))
                nc.tensor.matmul(psseg[:, :32], hnm[:, sl],
                                 bvohc[:, t * 32:(t + 1) * 32],
                                 start=(t == 0), stop=(t == NT - 1))
                if l < 3 and t % 8 == 7:
                    write_hnode(t // 8)

            # ---- AllGather h' right away (pre-BN); stats AR runs in parallel
            if l < 3:
                ag()

            # ---- BN stats -> AllReduce -> affine constants
            dtile = wk.tile([128, 128], F32, tag="dtile")
            nc.vector.tensor_tensor(dtile[:], psg[:, :128], IDENTf, op=ALU.mult)
            stat = wk.tile([128, 2], F32, tag="stat")
            nc.vector.reduce_sum(stat[:, 1:2], dtile[:], axis=AX.X)
            nc.vector.tensor_copy(stat[:, 0:1], psm[:, :])
            nc.sync.dma_start(d_stat_in.ap(), stat[:])
            CC("AllReduce", ALU.add, [d_stat_in.ap().opt()], [d_stat_out.ap().opt()])
            gstat = wk.tile([128, 2], F32, tag="gstat")
            nc.sync.dma_start(gstat[:], d_stat_out.ap())
            mean = wk.tile([128, 1], F32, tag="mean")
            nc.vector.tensor_scalar_mul(mean[:], gstat[:, 0:1], 1.0 / N)
            var = wk.tile([128, 1], F32, tag="var")
            nc.vector.tensor_scalar_mul(var[:], gstat[:, 1:2], 1.0 / N)
            m2 = wk.tile([128, 1], F32, tag="m2")
            nc.vector.tensor_tensor(m2[:], mean[:], mean[:], op=ALU.mult)
            nc.vector.tensor_tensor(var[:], var[:], m2[:], op=ALU.subtract)
            std = wk.tile([128, 1], F32, tag="std")
            nc.scalar.activation(std[:], var[:], AF.Sqrt, bias=eps_t[:])
            rstd = wk.tile([128, 1], F32, tag="rstd")
            nc.vector.reciprocal(rstd[:], std[:])
            a_col = wk.tile([128, 1], F32, tag="acol")
            nc.vector.tensor_tensor(a_col[:], FB(f'bng_{l}'), rstd[:], op=ALU.mult)
            c_col = wk.tile([128, 1], F32, tag="ccol")
            nc.vector.tensor_tensor(c_col[:], mean[:], a_col[:], op=ALU.mult)
            nc.vector.tensor_tensor(c_col[:], FB(f'bnb_{l}'), c_col[:], op=ALU.subtract)
            # csum += c
            if l == 0:
                nc.vector.tensor_copy(csum[:], c_col[:])
            else:
                nc.vector.tensor_tensor(csum[:], csum[:], c_col[:], op=ALU.add)
            # gp_accT += a (.) seg-sum^T(h')
            if l == 0:
                nc.vector.tensor_scalar_mul(gp_accT[:], psseg[:, :32], a_col[:])
            else:
                nc.vector.scalar_tensor_tensor(gp_accT[:], psseg[:, :32], a_col[:],
                                               gp_accT[:], op0=ALU.mult, op1=ALU.add)
            # np_fm += a (.) h'^T  (c handled via csum folded into policy bias)
            for t in range(NT):
                sl = slice(t * 128, (t + 1) * 128)
                psf = psT.tile([128, 128], F32, tag="PT", name="npT")
                nc.tensor.matmul(psf[:, :128], hnm[:, sl], identb,
                                 start=True, stop=True)
                if l == 0:
                    nc.vector.tensor_scalar_mul(np_fm[:, sl], psf[:, :128], a_col[:])
                else:
                    nc.vector.scalar_tensor_tensor(np_fm[:, sl], psf[:, :128],
                                                   a_col[:], np_fm[:, sl],
                                                   op0=ALU.mult, op1=ALU.add)
            if l < 3:
                # w1' = diag(a) @ w1_{l+1}
                w1s = w1p.tile([128, 128], BF16, tag="w1s", name=f"w1s{l + 1}")
                nc.vector.tensor_scalar_mul(w1s[:], BB(f'gw1_{l}', 128), a_col[:])
                w1s_cur[0] = w1s[:]
                # c' = c / a as bf16 row
                ra = wk.tile([128, 1], F32, tag="ra")
                nc.vector.reciprocal(ra[:], a_col[:])
                cp = wk.tile([128, 1], F32, tag="cp")
                nc.vector.tensor_tensor(cp[:], c_col[:], ra[:], op=ALU.mult)
                cpb = wk.tile([128, 1], BF16, tag="cpb")
                nc.vector.tensor_copy(cpb[:], cp[:])
                psr3 = psT.tile([128, 128], F32, tag="PT", name=f"cprow{l}")
                nc.tensor.matmul(psr3[0:1, :128], cpb[:], identb, start=True, stop=True)
                cprow = w1p.tile([1, 128], BF16, tag="cprow", name=f"cpr{l + 1}")
                nc.vector.tensor_copy(cprow[:], psr3[0:1, :128])
                cprow_cur[0] = cprow[:]

        DUMP(1, hnm, cast=True)
        DUMP(2, np_fm, cast=True)
        if stage <= 2:
            for p in (psT, psG, psM, psA, w1p, wk, gp, pers):
                p.release()
            return nc

        # ---------------- gpool via ReduceScatter (gp_accT accumulated per layer)
        psq0 = psT.tile([128, 128], F32, tag="PT", name="gpinT")
        nc.tensor.transpose(psq0[:32, :128], gp_accT[:], IDENTf)
        gpin = wk.tile([32, H], F32, tag="gpin")
        nc.vector.tensor_copy(gpin[:, :H], psq0[:32, :128])
        nc.sync.dma_start(d_gp_in.ap(), gpin[:])
        CC("ReduceScatter", ALU.add, [d_gp_in.ap().opt()], [d_gp_out.ap().opt()])
        gpl = wk.tile([GPC, H], F32, tag="gpl")
        nc.sync.dma_start(gpl[:], d_gp_out.ap())
        gsc = wk.tile([GPC, H], F32, tag="gsc")
        nc.vector.tensor_scalar_mul(gsc[:], gpl[:, :H], FB('rgcnt', 1, rows=GPC))
        # gpT [f, g] = gsc^T via matmul with IDENT[:4,:4]
        psq = psT.tile([128, 128], F32, tag="PT", name="gpT")
        nc.tensor.matmul(psq[:, :GPC], gsc[:], IDENTf[:GPC, :GPC],
                         start=True, stop=True)
        gpT = wk.tile([128, GPC], F32, tag="gpT")
        nc.vector.tensor_copy(gpT[:], psq[:, :GPC])
        # gpool_true = segmean(np) + csum (BN c-terms deferred from np)
        nc.vector.tensor_tensor(gpT[:], gpT[:],
                                csum[:].to_broadcast([128, GPC]), op=ALU.add)
        # policy-0 per-graph bias cols: pb10 + w1a^T csum + w1b^T gpool_g
        psb = psT.tile([128, 128], F32, tag="PT", name="polbias")
        nc.tensor.matmul(psb[:, 0:GPC], FB('pw10b', 128), gpT[:],
                         start=True, stop=True)
        psc1 = psT.tile([128, 128], F32, tag="PT", name="polbias1")
        nc.tensor.matmul(psc1[:, 0:1], FB('pw10a', 128), csum[:],
                         start=True, stop=True)
        c1sb = wk.tile([128, 1], F32, tag="c1sb")
        nc.vector.tensor_copy(c1sb[:], psc1[:, 0:1])
        bias4 = pers.tile([128, GPC], F32, tag="bias4")
        nc.vector.tensor_tensor(bias4[:], psb[:, 0:GPC],
                                c1sb[:].to_broadcast([128, GPC]), op=ALU.add)
        nc.vector.tensor_scalar_add(bias4[:], bias4[:], FB('pb10'))

        # ---------------- pi zero-fill (DMA idle in the endgame region).
        # zft = np_fm * 0 gives the fill a dependency on the last GIN layer so
        # the scheduler cannot hoist this 47us DMA into the busy layer phase.
        zft = pers.tile([128, 512], F32, tag="zf")
        nc.vector.tensor_scalar_mul(zft[:], bias4[:, 0:1].to_broadcast([128, 512]), 0.0)
        nreps = (GPC * NNODES * NNODES) // (128 * 512)
        pi_fill_v = p_pi.ap().rearrange("g (a p e) -> (g a) p e", p=128, e=512
                                        ).rearrange("x p e -> p x e")
        nch = nreps // 8
        for fch in range(8):
            nc.gpsimd.dma_start(
                pi_fill_v[:, fch * nch:(fch + 1) * nch, :],
                zft[:].unsqueeze(1).to_broadcast([128, nch, 512]))


        # ---------------- policy MLP (feature-major, fp32r)
        # zfm aliases np_fm's buffer (np_fm fully consumed by the first pass)
        t1 = pers.tile([128, NL], BF16, tag="t1")

        def R(ap_):
            return ap_.bitcast(F32R)

        for j in range(NL // 512):
            js = slice(j * 512, (j + 1) * 512)
            g = j // 2
            ps1 = psA.tile([128, 512], F32, tag="PP", name="ps1")
            nc.tensor.matmul(ps1[:, :512], BB('pw10ab', 128), np_fm[:, js],
                             start=True, stop=True)
            nc.scalar.activation(t1[:, js], ps1[:, :512], AF.Tanh,
                                 bias=bias4[:, g:g + 1])
        zfm = pers.tile([128, NL], BF16, tag="npfm", name="zfm")
        for j in range(NL // 512):
            js = slice(j * 512, (j + 1) * 512)
            ps2 = psA.tile([128, 512], F32, tag="PP", name="ps2")
            nc.tensor.matmul(ps2[:, :512], BB('pw20b', 128), t1[:, js],
                             start=True, stop=True)
            nc.vector.tensor_scalar_add(zfm[:, js], ps2[:, :512], FB('pb20'))
        znm = pers.tile([128, NL], F32, tag="znm", name="znm")
        d_z_v = d_z.ap().rearrange("(t p) f -> p t f", p=128)
        znm_v = znm[:].rearrange("p (t f) -> p t f", f=128)
        for j in range(NL // 512):
            js = slice(j * 512, (j + 1) * 512)
            ps1 = psA.tile([128, 512], F32, tag="PP", name="ps1")
            nc.tensor.matmul(ps1[:, :512], BB('pw1b_0', 128), zfm[:, js],
                             start=True, stop=True)
            nc.scalar.activation(t1[:, js], ps1[:, :512], AF.Tanh,
                                 bias=FB('pb1_0'))
        for j in range(NL // 512):
            js = slice(j * 512, (j + 1) * 512)
            ps2 = psA.tile([128, 512], F32, tag="PP", name="ps2")
            nc.tensor.matmul(ps2[:, :512], BB('pw2b_0', 128), t1[:, js],
                             start=True, stop=True)
            nc.vector.tensor_scalar_add(zfm[:, js], ps2[:, :512], FB('pb2_0'))
        for j in range(NL // 512):
            js = slice(j * 512, (j + 1) * 512)
            ps1 = psA.tile([128, 512], F32, tag="PP", name="ps1")
            nc.tensor.matmul(ps1[:, :512], BB('pw1b_1', 128), zfm[:, js],
                             start=True, stop=True)
            nc.scalar.activation(t1[:, js], ps1[:, :512], AF.Tanh,
                                 bias=FB('pb1_1'))
        zf32 = pers.tile([128, NL], F32, tag="zf32")
        for j in range(NL // 512):
            # final layer: z kept in f32 (feeds the softmax dots); transpose
            # tiles to node-major as soon as each chunk lands
            js = slice(j * 512, (j + 1) * 512)
            ps2 = psA.tile([128, 512], F32, tag="PP", name="ps2")
            nc.tensor.matmul(ps2[:, :512], BB('pw2b_1', 128), t1[:, js],
                             start=True, stop=True)
            nc.vector.tensor_scalar_add(zf32[:, js], ps2[:, :512], FB('pb2_1'))
            for t in range(4 * j, 4 * j + 4):
                sl = slice(t * 128, (t + 1) * 128)
                pst = psT.tile([128, 128], F32, tag="PT", name="zT")
                nc.tensor.transpose(pst[:, :128], zf32[:, sl], IDENTf)
                nc.vector.tensor_copy(znm[:, sl], pst[:, :128])
            nc.sync.dma_start(d_z_v[:, 4 * j:4 * j + 4, :],
                              znm_v[:, 4 * j:4 * j + 4, :])

        DUMP(4, zf32)
        if stage <= 4:
            for p in (psT, psG, psM, psA, w1p, wk, gp, pers):
                p.release()
            return nc

        # ---------------- masked sparse softmax
        mzr = pers.tile([128, MSLOT], F32, tag="mzr")
        nc.gpsimd.dma_gather(
            mzr[:].rearrange("p (c e) -> p c e", e=128),
            d_z[:, :], mridx_sb[:], MSLOT, MSLOT, 128, single_packet=False)
        mzc = pers.tile([128, MSLOT], F32, tag="mzc")
        nc.gpsimd.dma_gather(
            mzc[:].rearrange("p (c e) -> p c e", e=128),
            d_z[:, :], mcidx_sb[:], MSLOT, MSLOT, 128, single_packet=False)
        prod = pers.tile([128, MSLOT], F32, tag="pay", name="prod")
        nc.vector.tensor_tensor(prod[:], mzr[:], mzc[:], op=ALU.mult)
        dots = wk.tile([128, GPC], F32, tag="dots")
        nc.vector.reduce_sum(dots[:], prod[:].rearrange("p (g e) -> p g e", e=128),
                             axis=AX.X)
        em = wk.tile([128, GPC], F32, tag="em")
        nc.scalar.activation(em[:], dots[:], AF.Exp, bias=zcol[:])
        nc.vector.tensor_tensor(em[:], em[:], FB('mvalid', GPC), op=ALU.mult)
        psumg = psT.tile([128, 128], F32, tag="PT", name="psumg")
        nc.tensor.matmul(psumg[0:1, :GPC], onescf, em[:], start=True, stop=True)
        invT = wk.tile([1, GPC], F32, tag="invT")
        nc.vector.reciprocal(invT[:], psumg[0:1, :GPC])
        prep3 = psT.tile([128, 128], F32, tag="PT", name="prep3")
        nc.tensor.matmul(prep3[:, :GPC], onesrf, invT[:],
                         start=True, stop=True)
        vv = wk.tile([128, GPC], F32, tag="vv")
        nc.vector.tensor_tensor(vv[:], em[:], prep3[:, :GPC], op=ALU.mult)

        # scatter payload: slot (k, g) -> one-hot(col) * vv; one call per
        # row-occurrence level so no two descriptors of a call share a row
        for sc in range(NSC):
            pay = pers.tile([128, GPC * 128], F32, tag="pay", name=f"pay{sc}")
            for g in range(GPC):
                nc.vector.tensor_scalar(
                    pay[:, g * 128:(g + 1) * 128], J128f,
                    FB('mcolmod', NSC * GPC)[:, sc * GPC + g:sc * GPC + g + 1],
                    vv[:, g:g + 1], op0=ALU.is_equal, op1=ALU.mult)
            nc.gpsimd.dma_scatter_add(
                p_pi.ap().rearrange("g (r e) -> (g r) e", e=128),
                pay[:].rearrange("p (g e) -> p g e", e=128),
                msrow_sb[:, sc * (MSLOT // 16):(sc + 1) * (MSLOT // 16)],
                MSLOT, MSLOT, 128, single_packet=False)

        for p in (psT, psG, psM, psA, w1p, wk, gp, pers):
            p.release()
    return nc


# ---------------------------------------------------------------- numpy ref
def ref_np(inputs):
    x = np.asarray(inputs['x'], np.float64)
    ei = np.asarray(inputs['edge_index'])
    bv = np.asarray(inputs['batch_vec'])
    mr = np.asarray(inputs['mask_rows'])
    mc = np.asarray(inputs['mask_cols'])
    src, dst = ei[0], ei[1]
    deg = np.maximum(np.bincount(dst, minlength=N), 1.0)[:, None]

    def gin(h, w1, b1, w2, b2):
        agg = np.zeros_like(h)
        np.add.at(agg, dst, h[src])
        z = h + agg / deg
        return np.maximum(z @ w1 + b1, 0.0) @ w2 + b2

    def bn(h, g, b):
        mu = h.mean(0)
        var = h.var(0)
        return (h - mu) / np.sqrt(var + 1e-5) * g + b

    counts = np.maximum(np.bincount(bv, minlength=B), 1.0)[:, None]
    h = bn(np.maximum(gin(x, inputs['gin0_w1'], inputs['gin0_b1'],
                          inputs['gin0_w2'], inputs['gin0_b2']), 0.0),
           inputs['bn_gamma'][0], inputs['bn_beta'][0])
    node_pool = h.copy()
    gpool = np.zeros((B, H))
    np.add.at(gpool, bv, h)
    gpool = gpool / counts
    for l in range(3):
        h = bn(np.maximum(gin(h, inputs['gin_w1'][l], inputs['gin_b1'][l],
                              inputs['gin_w2'][l], inputs['gin_b2'][l]), 0.0),
               inputs['bn_gamma'][l + 1], inputs['bn_beta'][l + 1])
        node_pool += h
        gp2 = np.zeros((B, H))
        np.add.at(gp2, bv, h)
        gpool += gp2 / counts
    aug = np.concatenate([node_pool, np.repeat(gpool, NNODES, axis=0)],
                         axis=-1).reshape(B, NNODES, 2 * H)

    def pol(z, w1, b1, w2, b2):
        return np.tanh(z @ w1 + b1) @ w2 + b2

    z = pol(aug, inputs['pol0_w1'], inputs['pol0_b1'],
            inputs['pol0_w2'], inputs['pol0_b2'])
    for l in range(2):
        z = pol(z, inputs['pol_w1'][l], inputs['pol_b1'][l],
                inputs['pol_w2'][l], inputs['pol_b2'][l])
    score = np.einsum('bnd,bmd->bnm', z, z)
    mask = np.ones((B * NNODES, NNODES), bool)
    mask[mr, mc] = False
    score = np.where(mask.reshape(B, NNODES, NNODES), -np.inf, score)
    sc = score.reshape(B, -1)
    sc = sc - sc.max(-1, keepdims=True)
    e = np.exp(sc)
    return (e / e.sum(-1, keepdims=True)).astype(np.float32)


_CACHE = {}


def kernel(**inputs):
    in_maps, meta = prep(inputs)
    key = (meta['CAP'], meta['NSC'])
    if key not in _CACHE:
        nc = build(meta)
        nc.compile()
        _CACHE[key] = build_exec(nc, NCORES)
    run = _CACHE[key]
    res, times = run(in_maps, repeats=1)
    kernel.last_times = times
    return np.concatenate([res[c]["pi"].reshape(GPC, -1) for c in range(NCORES)], 0)
